# revision 11
# baseline (speedup 1.0000x reference)
"""LoCon1d (position-specific conv1d) Trainium2 kernel.

out[b,o,s] = sum_{c,k} xpad[b,c,s+k] * w[o,c,s,k] + bias[o,s]
shapes: x (16,64,1024) f32, w (64,64,1024,3) f32, bias (64,1024) f32.

Sharding: sequence-parallel over 8 cores, 128 positions each.
Per-core mapping: positions split into two half-blocks (j, 64+j) packed
block-diagonally into the 128-partition contraction dim of the PE:
  stationary lhsT [128, 32]: rows 0:64 = x window (c) for pos j,
    cols 0:16; rows 64:128 = x window for pos 64+j, cols 16:32 (zeros
    elsewhere, baked in on host).
  moving rhs [128, 64]: rows 0:64 = w[o, c, j, k], rows 64:128 =
    w[o, c, 64+j, k] -> psum[0:16,o] = out(pos j), psum[16:32,o] =
    out(pos 64+j). 3 taps accumulate in PSUM.
All device tensors are host-side relayouts so DMAs are contiguous.
"""

import numpy as np

import concourse.bass as bass
import concourse.mybir as mybir
import concourse.tile as tile
from concourse import bacc, bass_utils

N_CORES = 8
B, CIN, COUT, S, K = 16, 64, 64, 1024, 3
SC = S // N_CORES          # positions per core (128)
H = SC // 2                # half-block (64)
JB = 16                    # position chunks per half-block
JI = H // JB               # positions per chunk (4)
TW = H + K - 1             # x window length per half-block (66)
XCH = 1                    # xr DMA split (t-dim chunks)

_DT = {"f32": mybir.dt.float32, "bf16": mybir.dt.bfloat16,
       "f16": mybir.dt.float16}

DTYPE = "f16"


def _np_dt(dt):
    if dt == "bf16":
        import ml_dtypes
        return ml_dtypes.bfloat16
    if dt == "f16":
        return np.float16
    return np.float32


def build_bass(dtype=DTYPE):
    dt = _DT[dtype]
    nc = bacc.Bacc("TRN2", target_bir_lowering=False, debug=False,
                   num_devices=N_CORES)
    xr = nc.dram_tensor("xr", [128, TW, 32], dt, kind="ExternalInput")
    wr = nc.dram_tensor("wr", [128, JB, JI, K, COUT], dt, kind="ExternalInput")
    br = nc.dram_tensor("br", [32, H, COUT], mybir.dt.float32,
                        kind="ExternalInput")
    out = nc.dram_tensor("out", [32, H, COUT], mybir.dt.float32,
                         kind="ExternalOutput")

    with tile.TileContext(nc) as tc:
        with (
            tc.tile_pool(name="xpool", bufs=1) as xpool,
            tc.tile_pool(name="wpool", bufs=JB) as wpool,
            tc.tile_pool(name="bpool", bufs=1) as bpool,
            tc.tile_pool(name="opool", bufs=4) as opool,
            tc.tile_pool(name="psum", bufs=8, space="PSUM") as pspool,
        ):
            # x first (every matmul needs it), in t-chunks so early
            # windows land quickly; then weight chunks; bias off-path.
            xr_sb = xpool.tile([128, TW, 32], dt)
            step = (TW + XCH - 1) // XCH
            for c in range(XCH):
                t0, t1 = c * step, min((c + 1) * step, TW)
                nc.sync.dma_start(out=xr_sb[:, t0:t1, :],
                                  in_=xr.ap()[:, t0:t1, :])
            br_sb = bpool.tile([32, H, COUT], mybir.dt.float32)
            nc.gpsimd.dma_start(out=br_sb[:, :, :], in_=br.ap())
            w_t = []
            for jb in range(JB):
                wt = wpool.tile([128, JI, K, COUT], dt, tag="wt")
                nc.sync.dma_start(out=wt[:, :, :, :],
                                  in_=wr.ap()[:, jb, :, :, :])
                w_t.append(wt)

            for jb in range(JB):
                o_t = opool.tile([32, JI, COUT], mybir.dt.float32, tag="ot")
                for ji in range(JI):
                    j = jb * JI + ji
                    ps = pspool.tile([32, COUT], mybir.dt.float32, tag="ps")
                    for k in range(K):
                        nc.tensor.matmul(
                            ps[:, :],
                            lhsT=xr_sb[:, j + k, :],
                            rhs=w_t[jb][:, ji, k, :],
                            start=(k == 0),
                            stop=(k == K - 1),
                        )
                    nc.vector.tensor_add(
                        out=o_t[:, ji, :], in0=ps[:, :],
                        in1=br_sb[:, j, :],
                    )
                nc.sync.dma_start(out=out.ap()[:, jb * JI:(jb + 1) * JI, :],
                                  in_=o_t[:, :, :])
    nc.compile()
    return nc


def prep_inputs(input, weight, bias, dtype=DTYPE):
    """Host-side shard + relayout. Returns list of per-core input dicts."""
    npdt = _np_dt(dtype)
    xpad = np.pad(np.asarray(input, np.float32), ((0, 0), (0, 0), (1, 1)))
    w = np.asarray(weight, np.float32)
    bias = np.asarray(bias, np.float32)
    in_maps = []
    for i in range(N_CORES):
        s0 = i * SC
        # x: [p, b_ext, t] block-diagonal
        xa = xpad[:, :, s0:s0 + TW]             # (B, CIN, TW)
        xb = xpad[:, :, s0 + H:s0 + H + TW]
        xr = np.zeros((128, TW, 32), np.float32)
        xr[:64, :, :16] = xa.transpose(1, 2, 0)
        xr[64:, :, 16:] = xb.transpose(1, 2, 0)
        # w: [p(c+64*half), jb, ji, k, o]
        ws = w[:, :, s0:s0 + SC, :]             # (COUT, CIN, SC, K)
        wa = ws[:, :, :H, :].reshape(COUT, CIN, JB, JI, K)
        wb = ws[:, :, H:, :].reshape(COUT, CIN, JB, JI, K)
        wr = np.empty((128, JB, JI, K, COUT), np.float32)
        wr[:64] = wa.transpose(1, 2, 3, 4, 0)
        wr[64:] = wb.transpose(1, 2, 3, 4, 0)
        # bias: [b_ext, j, o] replicated over b
        bs = bias[:, s0:s0 + SC]                # (COUT, SC)
        br = np.empty((32, H, COUT), np.float32)
        br[:16] = bs[:, :H].T[None]
        br[16:] = bs[:, H:].T[None]
        in_maps.append({
            "xr": np.ascontiguousarray(xr.astype(npdt)),
            "wr": np.ascontiguousarray(wr.astype(npdt)),
            "br": np.ascontiguousarray(br),
        })
    return in_maps


def assemble_output(results):
    full = np.empty((B, COUT, S), np.float32)
    for i, r in enumerate(results):
        s0 = i * SC
        oc = r["out"]                            # (32, H, COUT)
        full[:, :, s0:s0 + H] = oc[:16].transpose(0, 2, 1)
        full[:, :, s0 + H:s0 + SC] = oc[16:].transpose(0, 2, 1)
    return full


_CACHED = {}


def run(inputs, dtype=DTYPE, trace=False):
    if dtype not in _CACHED:
        _CACHED[dtype] = build_bass(dtype)
    nc = _CACHED[dtype]
    in_maps = prep_inputs(inputs["input"], inputs["weight"], inputs["bias"],
                          dtype)
    res = bass_utils.run_bass_kernel_spmd(
        nc, in_maps, core_ids=list(range(N_CORES)), trace=trace)
    return assemble_output(res.results), res


def kernel(input, weight, bias):
    out, _ = run({"input": input, "weight": weight, "bias": bias},
                 trace=False)
    return out


# revision 12
# speedup vs baseline: 1.0287x; 1.0287x over previous
"""LoCon1d (position-specific conv1d) Trainium2 kernel.

out[b,o,s] = sum_{c,k} xpad[b,c,s+k] * w[o,c,s,k] + bias[o,s]
shapes: x (16,64,1024) f32, w (64,64,1024,3) f32, bias (64,1024) f32.

Sharding: sequence-parallel over 8 cores, 128 positions each.
Per-core mapping: positions split into two half-blocks (j, 64+j) packed
block-diagonally into the 128-partition contraction dim of the PE:
  stationary lhsT [128, 32]: rows 0:64 = x window (c) for pos j,
    cols 0:16; rows 64:128 = x window for pos 64+j, cols 16:32 (zeros
    elsewhere, baked in on host).
  moving rhs [128, 64]: rows 0:64 = w[o, c, j, k], rows 64:128 =
    w[o, c, 64+j, k] -> psum[0:16,o] = out(pos j), psum[16:32,o] =
    out(pos 64+j). 3 taps accumulate in PSUM.
All device tensors are host-side relayouts so DMAs are contiguous.
"""

import numpy as np

import concourse.bass as bass
import concourse.mybir as mybir
import concourse.tile as tile
from concourse import bacc, bass_utils

N_CORES = 8
B, CIN, COUT, S, K = 16, 64, 64, 1024, 3
SC = S // N_CORES          # positions per core (128)
H = SC // 2                # half-block (64)
JB = 16                    # position chunks per half-block
JI = H // JB               # positions per chunk (4)
TW = H + K - 1             # x window length per half-block (66)
XCH = 1                    # xr DMA split (t-dim chunks)

_DT = {"f32": mybir.dt.float32, "bf16": mybir.dt.bfloat16,
       "f16": mybir.dt.float16}

DTYPE = "f16"


def _np_dt(dt):
    if dt == "bf16":
        import ml_dtypes
        return ml_dtypes.bfloat16
    if dt == "f16":
        return np.float16
    return np.float32


def build_bass(dtype=DTYPE):
    dt = _DT[dtype]
    nc = bacc.Bacc("TRN2", target_bir_lowering=False, debug=False,
                   num_devices=N_CORES)
    xr = nc.dram_tensor("xr", [128, TW, 32], dt, kind="ExternalInput")
    wr = nc.dram_tensor("wr", [128, JB, JI, K, COUT], dt, kind="ExternalInput")
    br = nc.dram_tensor("br", [32, H, COUT], mybir.dt.float32,
                        kind="ExternalInput")
    out = nc.dram_tensor("out", [32, H, COUT], mybir.dt.float32,
                         kind="ExternalOutput")

    with tile.TileContext(nc) as tc:
        with (
            tc.tile_pool(name="xpool", bufs=1) as xpool,
            tc.tile_pool(name="wpool", bufs=JB) as wpool,
            tc.tile_pool(name="bpool", bufs=1) as bpool,
            tc.tile_pool(name="opool", bufs=4) as opool,
            tc.tile_pool(name="psum", bufs=8, space="PSUM") as pspool,
        ):
            # x first (every matmul needs it), in t-chunks so early
            # windows land quickly; then weight chunks; bias off-path.
            xr_sb = xpool.tile([128, TW, 32], dt)
            step = (TW + XCH - 1) // XCH
            for c in range(XCH):
                t0, t1 = c * step, min((c + 1) * step, TW)
                nc.sync.dma_start(out=xr_sb[:, t0:t1, :],
                                  in_=xr.ap()[:, t0:t1, :])
            br_sb = bpool.tile([32, H, COUT], mybir.dt.float32)
            nc.gpsimd.dma_start(out=br_sb[:, :, :], in_=br.ap())
            w_t = []
            for jb in range(JB):
                wt = wpool.tile([128, JI, K, COUT], dt, tag="wt")
                eng = nc.sync if jb % 2 == 0 else nc.gpsimd
                eng.dma_start(out=wt[:, :, :, :],
                              in_=wr.ap()[:, jb, :, :, :])
                w_t.append(wt)

            for jb in range(JB):
                o_t = opool.tile([32, JI, COUT], mybir.dt.float32, tag="ot")
                for ji in range(JI):
                    j = jb * JI + ji
                    ps = pspool.tile([32, COUT], mybir.dt.float32, tag="ps")
                    for k in range(K):
                        nc.tensor.matmul(
                            ps[:, :],
                            lhsT=xr_sb[:, j + k, :],
                            rhs=w_t[jb][:, ji, k, :],
                            start=(k == 0),
                            stop=(k == K - 1),
                        )
                    nc.vector.tensor_add(
                        out=o_t[:, ji, :], in0=ps[:, :],
                        in1=br_sb[:, j, :],
                    )
                nc.sync.dma_start(out=out.ap()[:, jb * JI:(jb + 1) * JI, :],
                                  in_=o_t[:, :, :])
    nc.compile()
    return nc


def prep_inputs(input, weight, bias, dtype=DTYPE):
    """Host-side shard + relayout. Returns list of per-core input dicts."""
    npdt = _np_dt(dtype)
    xpad = np.pad(np.asarray(input, np.float32), ((0, 0), (0, 0), (1, 1)))
    w = np.asarray(weight, np.float32)
    bias = np.asarray(bias, np.float32)
    in_maps = []
    for i in range(N_CORES):
        s0 = i * SC
        # x: [p, b_ext, t] block-diagonal
        xa = xpad[:, :, s0:s0 + TW]             # (B, CIN, TW)
        xb = xpad[:, :, s0 + H:s0 + H + TW]
        xr = np.zeros((128, TW, 32), np.float32)
        xr[:64, :, :16] = xa.transpose(1, 2, 0)
        xr[64:, :, 16:] = xb.transpose(1, 2, 0)
        # w: [p(c+64*half), jb, ji, k, o]
        ws = w[:, :, s0:s0 + SC, :]             # (COUT, CIN, SC, K)
        wa = ws[:, :, :H, :].reshape(COUT, CIN, JB, JI, K)
        wb = ws[:, :, H:, :].reshape(COUT, CIN, JB, JI, K)
        wr = np.empty((128, JB, JI, K, COUT), np.float32)
        wr[:64] = wa.transpose(1, 2, 3, 4, 0)
        wr[64:] = wb.transpose(1, 2, 3, 4, 0)
        # bias: [b_ext, j, o] replicated over b
        bs = bias[:, s0:s0 + SC]                # (COUT, SC)
        br = np.empty((32, H, COUT), np.float32)
        br[:16] = bs[:, :H].T[None]
        br[16:] = bs[:, H:].T[None]
        in_maps.append({
            "xr": np.ascontiguousarray(xr.astype(npdt)),
            "wr": np.ascontiguousarray(wr.astype(npdt)),
            "br": np.ascontiguousarray(br),
        })
    return in_maps


def assemble_output(results):
    full = np.empty((B, COUT, S), np.float32)
    for i, r in enumerate(results):
        s0 = i * SC
        oc = r["out"]                            # (32, H, COUT)
        full[:, :, s0:s0 + H] = oc[:16].transpose(0, 2, 1)
        full[:, :, s0 + H:s0 + SC] = oc[16:].transpose(0, 2, 1)
    return full


_CACHED = {}


def run(inputs, dtype=DTYPE, trace=False):
    if dtype not in _CACHED:
        _CACHED[dtype] = build_bass(dtype)
    nc = _CACHED[dtype]
    in_maps = prep_inputs(inputs["input"], inputs["weight"], inputs["bias"],
                          dtype)
    res = bass_utils.run_bass_kernel_spmd(
        nc, in_maps, core_ids=list(range(N_CORES)), trace=trace)
    return assemble_output(res.results), res


def kernel(input, weight, bias):
    out, _ = run({"input": input, "weight": weight, "bias": bias},
                 trace=False)
    return out


# revision 13
# speedup vs baseline: 1.0711x; 1.0413x over previous
"""LoCon1d (position-specific conv1d) Trainium2 kernel.

out[b,o,s] = sum_{c,k} xpad[b,c,s+k] * w[o,c,s,k] + bias[o,s]
shapes: x (16,64,1024) f32, w (64,64,1024,3) f32, bias (64,1024) f32.

Sharding: sequence-parallel over 8 cores, 128 positions each.
Per-core mapping: positions split into two half-blocks (j, 64+j) packed
block-diagonally into the 128-partition contraction dim of the PE:
  stationary lhsT [128, 32]: rows 0:64 = x window (c) for pos j,
    cols 0:16; rows 64:128 = x window for pos 64+j, cols 16:32 (zeros
    elsewhere, baked in on host).
  moving rhs [128, 64]: rows 0:64 = w[o, c, j, k], rows 64:128 =
    w[o, c, 64+j, k] -> psum[0:16,o] = out(pos j), psum[16:32,o] =
    out(pos 64+j). 3 taps accumulate in PSUM.
All device tensors are host-side relayouts so DMAs are contiguous.
"""

import numpy as np

import concourse.bass as bass
import concourse.mybir as mybir
import concourse.tile as tile
from concourse import bacc, bass_utils

N_CORES = 8
B, CIN, COUT, S, K = 16, 64, 64, 1024, 3
SC = S // N_CORES          # positions per core (128)
H = SC // 2                # half-block (64)
JB = 16                    # position chunks per half-block
JI = H // JB               # positions per chunk (4)
TW = H + K - 1             # x window length per half-block (66)
XCH = 2                    # xr DMA split (t-dim chunks)

_DT = {"f32": mybir.dt.float32, "bf16": mybir.dt.bfloat16,
       "f16": mybir.dt.float16}

DTYPE = "f16"


def _np_dt(dt):
    if dt == "bf16":
        import ml_dtypes
        return ml_dtypes.bfloat16
    if dt == "f16":
        return np.float16
    return np.float32


def build_bass(dtype=DTYPE):
    dt = _DT[dtype]
    nc = bacc.Bacc("TRN2", target_bir_lowering=False, debug=False,
                   num_devices=N_CORES)
    xr = nc.dram_tensor("xr", [128, TW, 32], dt, kind="ExternalInput")
    wr = nc.dram_tensor("wr", [128, JB, JI, K, COUT], dt, kind="ExternalInput")
    br = nc.dram_tensor("br", [32, H, COUT], mybir.dt.float32,
                        kind="ExternalInput")
    out = nc.dram_tensor("out", [32, H, COUT], mybir.dt.float32,
                         kind="ExternalOutput")

    with tile.TileContext(nc) as tc:
        with (
            tc.tile_pool(name="xpool", bufs=1) as xpool,
            tc.tile_pool(name="wpool", bufs=JB) as wpool,
            tc.tile_pool(name="bpool", bufs=1) as bpool,
            tc.tile_pool(name="opool", bufs=4) as opool,
            tc.tile_pool(name="psum", bufs=8, space="PSUM") as pspool,
        ):
            # x first (every matmul needs it), in t-chunks so early
            # windows land quickly; then weight chunks; bias off-path.
            xr_sb = xpool.tile([128, TW, 32], dt)
            step = (TW + XCH - 1) // XCH
            for c in range(XCH):
                t0, t1 = c * step, min((c + 1) * step, TW)
                nc.sync.dma_start(out=xr_sb[:, t0:t1, :],
                                  in_=xr.ap()[:, t0:t1, :])
            br_sb = bpool.tile([32, H, COUT], mybir.dt.float32)
            nc.gpsimd.dma_start(out=br_sb[:, :, :], in_=br.ap())
            w_t = []
            for jb in range(JB):
                wt = wpool.tile([128, JI, K, COUT], dt, tag="wt")
                eng = nc.sync if jb % 2 == 0 else nc.gpsimd
                eng.dma_start(out=wt[:, :, :, :],
                              in_=wr.ap()[:, jb, :, :, :])
                w_t.append(wt)

            for jb in range(JB):
                o_t = opool.tile([32, JI, COUT], mybir.dt.float32, tag="ot")
                for ji in range(JI):
                    j = jb * JI + ji
                    ps = pspool.tile([32, COUT], mybir.dt.float32, tag="ps")
                    for k in range(K):
                        nc.tensor.matmul(
                            ps[:, :],
                            lhsT=xr_sb[:, j + k, :],
                            rhs=w_t[jb][:, ji, k, :],
                            start=(k == 0),
                            stop=(k == K - 1),
                        )
                    nc.vector.tensor_add(
                        out=o_t[:, ji, :], in0=ps[:, :],
                        in1=br_sb[:, j, :],
                    )
                nc.sync.dma_start(out=out.ap()[:, jb * JI:(jb + 1) * JI, :],
                                  in_=o_t[:, :, :])
    nc.compile()
    return nc


def prep_inputs(input, weight, bias, dtype=DTYPE):
    """Host-side shard + relayout. Returns list of per-core input dicts."""
    npdt = _np_dt(dtype)
    xpad = np.pad(np.asarray(input, np.float32), ((0, 0), (0, 0), (1, 1)))
    w = np.asarray(weight, np.float32)
    bias = np.asarray(bias, np.float32)
    in_maps = []
    for i in range(N_CORES):
        s0 = i * SC
        # x: [p, b_ext, t] block-diagonal
        xa = xpad[:, :, s0:s0 + TW]             # (B, CIN, TW)
        xb = xpad[:, :, s0 + H:s0 + H + TW]
        xr = np.zeros((128, TW, 32), np.float32)
        xr[:64, :, :16] = xa.transpose(1, 2, 0)
        xr[64:, :, 16:] = xb.transpose(1, 2, 0)
        # w: [p(c+64*half), jb, ji, k, o]
        ws = w[:, :, s0:s0 + SC, :]             # (COUT, CIN, SC, K)
        wa = ws[:, :, :H, :].reshape(COUT, CIN, JB, JI, K)
        wb = ws[:, :, H:, :].reshape(COUT, CIN, JB, JI, K)
        wr = np.empty((128, JB, JI, K, COUT), np.float32)
        wr[:64] = wa.transpose(1, 2, 3, 4, 0)
        wr[64:] = wb.transpose(1, 2, 3, 4, 0)
        # bias: [b_ext, j, o] replicated over b
        bs = bias[:, s0:s0 + SC]                # (COUT, SC)
        br = np.empty((32, H, COUT), np.float32)
        br[:16] = bs[:, :H].T[None]
        br[16:] = bs[:, H:].T[None]
        in_maps.append({
            "xr": np.ascontiguousarray(xr.astype(npdt)),
            "wr": np.ascontiguousarray(wr.astype(npdt)),
            "br": np.ascontiguousarray(br),
        })
    return in_maps


def assemble_output(results):
    full = np.empty((B, COUT, S), np.float32)
    for i, r in enumerate(results):
        s0 = i * SC
        oc = r["out"]                            # (32, H, COUT)
        full[:, :, s0:s0 + H] = oc[:16].transpose(0, 2, 1)
        full[:, :, s0 + H:s0 + SC] = oc[16:].transpose(0, 2, 1)
    return full


_CACHED = {}


def run(inputs, dtype=DTYPE, trace=False):
    if dtype not in _CACHED:
        _CACHED[dtype] = build_bass(dtype)
    nc = _CACHED[dtype]
    in_maps = prep_inputs(inputs["input"], inputs["weight"], inputs["bias"],
                          dtype)
    res = bass_utils.run_bass_kernel_spmd(
        nc, in_maps, core_ids=list(range(N_CORES)), trace=trace)
    return assemble_output(res.results), res


def kernel(input, weight, bias):
    out, _ = run({"input": input, "weight": weight, "bias": bias},
                 trace=False)
    return out
